# revision 1
# baseline (speedup 1.0000x reference)
"""GatedLTMMemory kernel for 8 Trainium2 NeuronCores.

Data-parallel over the 4096 flattened (B,N) tokens: 512 tokens per core.
Memory-slot tables and weights are replicated. The reference's per-selected-slot
projections (137 GFLOP) are replaced by projecting the slot tables once and
running a masked full-softmax over all S slots (exactly equivalent math).

Precision plan (fp32 matmuls run at 1/4 PE rate; float32r/bf16 at full rate):
  exact fp32 : selection path (q projection, slot norms, scores) — the top-32
               boundary gaps are ~1e-6 so this path cannot be rounded.
  float32r   : Kp/Vp/qh projections, attention logits, Wo/Wout epilogue
               (~1.6e-4 measured on HW).
  bf16       : softmax weights w = exp(att)*mask and the value table Vp
               (~2e-3; the denominators come from the same w so it cancels).

Emission order is chosen so the DVE top-k overlaps the PE Kp/Vp/qh
projections. SBUF pool tags are allocated statically, so dead tensors donate
their slots to later tensors (chains are noted inline). Host passes
weights/tables pre-transposed (layout prep only; no FLOPs moved to host).
"""

import numpy as np

import concourse.bacc as bacc
import concourse.mybir as mybir
import concourse.tile as tile
from concourse.bass import ds, ts
from concourse.bass_utils import run_bass_kernel_spmd
from concourse.masks import make_identity

B, N, QD, D, S, H, K = 4, 1024, 320, 512, 1024, 8, 32
DH = D // H
EPS = 1e-5
P = 128
T = 512                       # tokens per core
NCORES = 8
NT = T // P                   # 4 token tiles
ND = D // P                   # 4 contraction chunks over D
NS = S // P                   # 8 slot tiles
NEG = -1e30
QD_TILES = [(0, 128), (128, 128), (256, 64)]

f32 = mybir.dt.float32
f32r = mybir.dt.float32r
bf16 = mybir.dt.bfloat16
AF = mybir.ActivationFunctionType
OP = mybir.AluOpType

_CACHE: dict = {}


def _build_nc():
    nc = bacc.Bacc("TRN2", target_bir_lowering=False, debug=False)

    dr = {}

    def din(name, shape, dt_):
        dr[name] = nc.dram_tensor(name, shape, dt_, kind="ExternalInput")

    din("queryT", (QD, T), f32)
    din("WqpT", (QD, D), f32)
    din("WqT", (D, D), f32r)
    din("WkT", (D, D), f32r)
    din("WvT", (D, D), f32r)
    din("WoT", (D, D), f32r)
    din("WoutT", (D, QD), f32r)
    din("memkT", (D, S), f32)
    din("memvT", (D, S), f32)
    din("ln_g", (D,), f32)
    din("ln_b", (D,), f32)
    din("bout", (384,), f32)
    out_dram = nc.dram_tensor("outT", (QD, T), f32, kind="ExternalOutput")

    with tile.TileContext(nc) as tc:
        with (
            tc.tile_pool(name="const", bufs=1) as const,
            tc.tile_pool(name="main", bufs=1) as main,
            tc.tile_pool(name="scr2", bufs=2) as scr2,
            tc.tile_pool(name="scr4", bufs=8) as scr4,
            tc.tile_pool(name="psA", bufs=2, space="PSUM") as psA,
            tc.tile_pool(name="psB", bufs=1, space="PSUM") as psB,
            tc.tile_pool(name="psmm", bufs=4, space="PSUM") as psmm,
            nc.allow_low_precision(reason="validated f32r/bf16 paths"),
        ):
            # ---------- constants ----------
            ident = const.tile([P, P], bf16, tag="ident")
            make_identity(nc, ident)
            ident_f = const.tile([P, P], f32, tag="ident_f")
            make_identity(nc, ident_f)
            ones_col = const.tile([P, 1], f32, tag="ones_col")
            nc.vector.memset(ones_col, 1.0)
            ones_row = const.tile([1, P], f32, tag="ones_row")
            nc.vector.memset(ones_row, 1.0)
            # f32r half-ones rows for per-head-pair broadcast matmuls
            halfsel = const.tile([1, 2 * P], f32, tag="halfsel")
            nc.vector.memset(halfsel, 0.0)
            nc.vector.memset(halfsel[0:1, 64:192], 1.0)
            halfsel_r = const.tile([1, 2 * P], f32r, tag="halfsel_r")
            nc.scalar.copy(halfsel_r[:], halfsel[:])
            # halfsel layout: [0:64]=0, [64:192]=1, [192:256]=0
            ones_row_r = halfsel_r[0:1, 64:192]  # [1,128] all ones
            selA = halfsel_r[0:1, 128:256]       # [1,128]: ones x64, zeros x64
            selB = halfsel_r[0:1, 0:128]         # [1,128]: zeros x64, ones x64
            eps_tab = const.tile([P, 1], f32, tag="eps_tab")
            nc.vector.memset(eps_tab, 1e-12)
            eps_ln = const.tile([1, 1], f32, tag="eps_ln")
            nc.vector.memset(eps_ln, EPS)

            # ---------- weight loads ----------
            def load_rows(name, cols, row_tiles, tags, dt_):
                tiles = []
                for (off, sz), tag in zip(row_tiles, tags):
                    t_ = main.tile([sz, cols], dt_, tag=tag, name=f"ld_{tag}")
                    nc.sync.dma_start(t_[:], dr[name].ap()[ds(off, sz), :])
                    tiles.append(t_)
                return tiles

            d_rows = [(i * P, P) for i in range(ND)]
            qryT = load_rows("queryT", T, QD_TILES, ["qry0", "qry1", "qry2"], f32)
            wqpT = load_rows("WqpT", D, QD_TILES, ["wqp0", "wqp1", "wqp2"], f32)

            g_sb = const.tile([P, ND], f32, tag="g")
            nc.sync.dma_start(g_sb[:], dr["ln_g"].ap().rearrange("(o p) -> p o", p=P))
            b_sb = const.tile([P, ND], f32, tag="b")
            nc.sync.dma_start(b_sb[:], dr["ln_b"].ap().rearrange("(o p) -> p o", p=P))
            bout_sb = const.tile([P, 3], f32, tag="bout")
            nc.sync.dma_start(bout_sb[:], dr["bout"].ap().rearrange("(o p) -> p o", p=P))

            ktiles = load_rows("memkT", S, d_rows, [f"t14_{i}" for i in range(ND)], f32)

            from concourse import bass_isa

            # ---------- qT[d, t] = Wqp @ query.T (exact fp32; f32r copy for qh) ----
            # emitted first so the PE has work while the tables normalize
            qTr_tags = ["qry0", "qry1", "qry2", "wqp0"]
            qT = []
            for dt_i in range(ND):
                t_ = main.tile([P, T], f32, tag=f"qt{dt_i}", name=f"q{dt_i}")
                ps = psmm.tile([P, T], f32, tag="mm")
                for c in range(3):
                    nc.tensor.matmul(
                        ps, lhsT=wqpT[c][:, ts(dt_i, P)], rhs=qryT[c][:],
                        start=(c == 0), stop=(c == 2),
                    )
                nc.scalar.copy(t_[:], ps)
                qT.append(t_)
            qTr = []
            for dt_i in range(ND):
                tr_ = main.tile([P, T], f32r, tag=qTr_tags[dt_i], name=f"qr{dt_i}")
                nc.vector.tensor_copy(tr_[:], qT[dt_i][:])
                qTr.append(tr_)

            wqT = load_rows("WqT", D, d_rows, [f"wq{i}" for i in range(ND)], f32r)
            wkT = load_rows("WkT", D, d_rows, [f"wkw{i}" for i in range(ND)], f32r)
            vtiles = load_rows("memvT", S, d_rows, [f"t58_{i}" for i in range(ND)], f32)
            wvT = load_rows("WvT", D, d_rows, [f"wvw{i}" for i in range(ND)], f32r)
            woT = load_rows("WoT", D, d_rows, [f"wo{i}" for i in range(ND)], f32r)
            woutT = load_rows("WoutT", QD, d_rows, [f"wu{i}" for i in range(ND)], f32r)

            # ---------- slot tables: l2-normalize in transposed layout ----------
            # keys (on the scores critical path): PE ones-matmul for the
            # partition sum-of-squares. vals (off critical path): GPSIMD
            # partition_all_reduce, whose output is replicated so the rescale
            # needs no broadcast matmul.
            def normalize_keys(tiles):
                ps_halves = []
                for half in range(2):
                    if half == 0:
                        ps_ssq = psA.tile([1, T], f32, tag="bc", name="ssq0")
                    else:
                        ps_ssq = psA.tile([1, T], f32, tag="ctx", name="ssq1")
                    for i in range(ND):
                        sq = scr2.tile([P, T], f32, tag="sq")
                        nc.scalar.square(sq, tiles[i][:, ds(half * T, T)])
                        nc.tensor.matmul(
                            ps_ssq, lhsT=ones_col, rhs=sq,
                            start=(i == 0), stop=(i == ND - 1),
                        )
                    ps_halves.append(ps_ssq)
                sd_row = main.tile([1, S], f32, tag="sdrow", name="sdr")
                for half in range(2):
                    nc.scalar.activation(
                        sd_row[:, ds(half * T, T)], ps_halves[half], AF.Sqrt,
                        bias=eps_tab[0:1, :],
                    )
                rsq_row = main.tile([1, S], f32, tag="rsqrow", name="rsq")
                nc.vector.reciprocal(rsq_row, sd_row)
                rsqB = main.tile([P, S], f32, tag="rsqB", name="rsqB")
                for half in range(2):
                    ps_b = psA.tile([P, T], f32, tag="bc")
                    nc.tensor.matmul(
                        ps_b, lhsT=ones_row, rhs=rsq_row[:, ds(half * T, T)],
                        start=True, stop=True,
                    )
                    nc.scalar.copy(rsqB[:, ds(half * T, T)], ps_b)
                for i in range(ND):
                    nc.vector.tensor_tensor(tiles[i][:], tiles[i][:], rsqB[:], OP.mult)
                return tiles

            def normalize_vals(tiles):
                sqsum = main.tile([P, S], f32, tag="rsqrow", name="sqs")
                for i in range(ND):
                    sq = main.tile([P, S], f32, tag=f"wk{i}", name=f"vsq{i}")
                    nc.scalar.square(sq[:], tiles[i][:])
                    if i == 0:
                        nc.gpsimd.tensor_copy(sqsum[:], sq[:])
                    else:
                        nc.gpsimd.tensor_tensor(sqsum[:], sqsum[:], sq[:], OP.add)
                rsq_full = main.tile([P, S], f32, tag="rsqB", name="rsqf")
                nc.gpsimd.partition_all_reduce(
                    rsq_full[:], sqsum[:], channels=P, reduce_op=bass_isa.ReduceOp.add
                )
                nc.scalar.activation(sqsum[:], rsq_full[:], AF.Sqrt, bias=eps_tab[:])
                nc.vector.reciprocal(rsq_full[:], sqsum[:])
                for i in range(ND):
                    nc.gpsimd.tensor_tensor(
                        tiles[i][:], tiles[i][:], rsq_full[:], OP.mult
                    )
                return tiles

            # keys; t14 slots chain: keysnT -> mask01
            keysnT = normalize_keys(ktiles)
            # rounded copy of keysnT for the f32r KpT matmul (scores keep fp32)
            ktr = []
            for i in range(ND):
                t_ = main.tile([P, S], f32r, tag=f"ktr{i}", name=f"ktr{i}")
                nc.vector.tensor_copy(t_[:], keysnT[i][:])
                ktr.append(t_)
            # vals; t58 slots chain: valsnT -> scores; wk: vals-sq -> topk scratch
            valsnT = normalize_vals(vtiles)
            vtr_tags = ["sdrow", "rsqrow", "rsqB", "vtr3"]
            vtr = []
            for i in range(ND):
                t_ = main.tile([P, S], f32r, tag=vtr_tags[i], name=f"vtr{i}")
                nc.vector.tensor_copy(t_[:], valsnT[i][:])
                vtr.append(t_)

            # ---------- scores[t, s] = q @ keysn.T (exact fp32), then top-32 ------
            sc = []
            for tt in range(NT):
                t_ = main.tile([P, S], f32, tag=f"t58_{tt}", name=f"sc{tt}")
                for half in range(2):
                    ps = psmm.tile([P, T], f32, tag="mm")
                    for dc in range(ND):
                        nc.tensor.matmul(
                            ps,
                            lhsT=qT[dc][:, ts(tt, P)],
                            rhs=keysnT[dc][:, ds(half * T, T)],
                            start=(dc == 0), stop=(dc == ND - 1),
                        )
                    nc.scalar.copy(t_[:, ds(half * T, T)], ps)
                sc.append(t_)

            # top-32 threshold per token row (4 rounds of max8), then bf16 mask
            mask01 = []
            for tt in range(NT):
                work = main.tile([P, S], f32, tag=f"wk{tt}", name=f"wk{tt}")
                cur = sc[tt]
                for r in range(4):
                    mx = main.tile([P, 8], f32, tag=f"mx{tt}_{r}", name=f"mx{tt}_{r}")
                    nc.vector.max(out=mx[:], in_=cur[:])
                    if r < 3:
                        nc.vector.match_replace(
                            out=work[:], in_to_replace=mx[:], in_values=cur[:],
                            imm_value=NEG,
                        )
                        cur = work
                m_ = main.tile([P, S], f32, tag=f"t14_{tt}", name=f"mk{tt}")
                nc.vector.tensor_scalar(
                    m_[:], sc[tt][:], mx[:, 7:8], None, op0=OP.is_ge
                )
                mask01.append(m_)

            # ---------- KpT[e, s] = Wk @ keysn.T  (f32r) ----------
            kpT = []
            for e in range(ND):
                t_ = main.tile([P, S], f32r, tag=f"kp{e}", name=f"kp{e}")
                for half in range(2):
                    ps = psmm.tile([P, T], f32, tag="mm")
                    for dc in range(ND):
                        nc.tensor.matmul(
                            ps,
                            lhsT=wkT[dc][:, ts(e, P)],
                            rhs=ktr[dc][:, ds(half * T, T)],
                            start=(dc == 0), stop=(dc == ND - 1),
                        )
                    nc.scalar.copy(t_[:, ds(half * T, T)], ps)
                kpT.append(t_)

            # ---------- Vp[s, 8 heads x (64 + ones)] = valsn @ Wv.T (bf16) --------
            vp = []
            for st in range(NS):
                t_ = main.tile([P, H, DH + 1], bf16, tag=f"vp{st}", name=f"vp{st}")
                nc.vector.memset(t_[:, :, DH : DH + 1], 1.0)
                ps = psmm.tile([P, D], f32, tag="mm")
                for dc in range(ND):
                    nc.tensor.matmul(
                        ps,
                        lhsT=vtr[dc][:, ts(st, P)],
                        rhs=wvT[dc][:],
                        start=(dc == 0), stop=(dc == ND - 1),
                    )
                nc.vector.tensor_copy(
                    t_[:, :, 0:DH], ps.rearrange("p (h e) -> p h e", h=H)
                )
                vp.append(t_)

            # ---------- qhT[e, t] = (Wq @ qT) / 8  (f32r) ----------
            qhT = []
            for e in range(ND):
                t_ = main.tile([P, T], f32r, tag=f"wvw{e}", name=f"qh{e}")
                ps = psmm.tile([P, T], f32, tag="mm")
                for dc in range(ND):
                    nc.tensor.matmul(
                        ps, lhsT=wqT[dc][:, ts(e, P)], rhs=qTr[dc][:],
                        start=(dc == 0), stop=(dc == ND - 1),
                    )
                nc.scalar.mul(t_[:], ps, 1.0 / np.sqrt(DH))
                qhT.append(t_)

            # ---------- masked attention over all S slots ----------
            # u (exp output) rotates over 20 dead slots: 4 retired qT tiles
            # plus 4x4 quarter-slices of the retired ktr tiles (free after the
            # KpT matmuls, i.e. BEFORE the top-k finishes). The first 20
            # units' logit matmuls + exps are EMITTED BEFORE the mask
            # transposes: they don't read the masks, so the PE/ACT stream
            # keeps working while the DVE finishes the top-k (a stalled
            # transpose would otherwise block everything behind it in PE
            # program order).
            u_singles = [
                main.tile([P, T], bf16, tag=f"qt{i}", name=f"us{i}")
                for i in range(4)
            ]
            u_quads = [
                main.tile([P, 4, T], bf16, tag=f"ktr{i}", name=f"uq{i}")
                for i in range(4)
            ]

            def u_slot(unit):
                m = unit % 20
                if m < 4:
                    return u_singles[m][:]
                m -= 4
                return u_quads[m // 4][:, m % 4, :]

            def att_exp(unit):
                h, c = unit // NS, unit % NS
                et, ro = h // 2, (h % 2) * 64
                ps_att = psmm.tile([P, T], f32, tag="mm", name=f"att{unit}")
                nc.tensor.matmul(
                    ps_att,
                    lhsT=kpT[et][ro : ro + DH, ts(c, P)],
                    rhs=qhT[et][ro : ro + DH, :],
                    start=True, stop=True,
                )
                u = u_slot(unit)
                nc.scalar.activation(u[:], ps_att, AF.Exp)
                return u

            PRE = 8
            u_pre = {unit: att_exp(unit) for unit in range(PRE)}

            # ---------- transpose the mask to [s, t] (bf16 PE transposes) ---------
            mT = []
            for j in range(NS):
                tag = ["qry0", "qry1", "qry2", "wqp0", "mT4", "mT5", "mT6", "mT7"][j]
                mT.append(main.tile([P, T], bf16, tag=tag, name=f"mT{j}"))
            for j in range(NS):
                ps_t = psA.tile([P, T], f32, tag="bc", name=f"pst{j}")
                for tt in range(NT):
                    nc.tensor.matmul(
                        ps_t[:, ts(tt, P)], lhsT=mask01[tt][:, ts(j, P)],
                        rhs=ident_f, is_transpose=True, skip_group_check=True,
                    )
                nc.scalar.copy(mT[j][:], ps_t)

            # wkw slots chain: WkT -> ctxT
            ctxT = [
                main.tile([P, T], f32, tag=f"wkw{dt_i}", name=f"cx{dt_i}")
                for dt_i in range(ND)
            ]
            for h in range(H):
                et, ro = h // 2, (h % 2) * 64
                if h % 2 == 0:
                    den_pair = scr2.tile([1, 2 * T], f32r, tag="den")
                ps_ctx = psA.tile([DH + 1, T], f32, tag="ctx")
                for c in range(NS):
                    unit = h * NS + c
                    u = u_pre.pop(unit) if unit in u_pre else att_exp(unit)
                    w = scr4.tile([P, T], bf16, tag="w")
                    nc.vector.tensor_tensor(w[:], u[:], mT[c][:], OP.mult)
                    nc.tensor.matmul(
                        ps_ctx, lhsT=vp[c][:, h, :], rhs=w[:],
                        start=(c == 0), stop=(c == NS - 1),
                    )
                nc.vector.tensor_copy(
                    ctxT[et][ro : ro + DH, :].bitcast(f32r), ps_ctx[0:DH, :]
                )
                # reciprocal straight from the PSUM denominator row — no copy
                nc.vector.reciprocal(
                    den_pair[0:1, ds((h % 2) * T, T)], ps_ctx[DH : DH + 1, :]
                )
                if h % 2 == 1:
                    # divide the head pair's ctx rows by their softmax denominators
                    ps_rb = psA.tile([P, T], f32, tag="bc")
                    nc.tensor.matmul(
                        ps_rb, lhsT=selA, rhs=den_pair[0:1, 0:T],
                        start=True, stop=False,
                    )
                    nc.tensor.matmul(
                        ps_rb, lhsT=selB, rhs=den_pair[0:1, T : 2 * T],
                        start=False, stop=True,
                    )
                    nc.vector.tensor_tensor(
                        ctxT[et][:].bitcast(f32r), ctxT[et][:], ps_rb, OP.mult
                    )

            # ---------- oT[e, t] = Wo @ ctx.T  (f32r); wq slots -> oT ----------
            oT = []
            for e in range(ND):
                t_ = main.tile([P, T], f32, tag=f"wq{e}", name=f"o{e}")
                ps = psmm.tile([P, T], f32, tag="mm")
                for dc in range(ND):
                    nc.tensor.matmul(
                        ps, lhsT=woT[dc][:, ts(e, P)],
                        rhs=ctxT[dc][:].bitcast(f32r),
                        start=(dc == 0), stop=(dc == ND - 1),
                    )
                nc.scalar.copy(t_[:], ps)
                oT.append(t_)

            # ---------- LayerNorm over e (partitions), stats via ones-matmul -----
            ps_mu = psA.tile([1, T], f32, tag="bc", name="psmu")
            ps_ms = psA.tile([1, T], f32, tag="ctx", name="psms")
            for dc in range(ND):
                sq = scr2.tile([P, T], f32, tag="sq")
                nc.scalar.square(sq, oT[dc][:])
                nc.tensor.matmul(
                    ps_mu, lhsT=ones_col, rhs=oT[dc][:],
                    start=(dc == 0), stop=(dc == ND - 1),
                )
                nc.tensor.matmul(
                    ps_ms, lhsT=ones_col, rhs=sq[:],
                    start=(dc == 0), stop=(dc == ND - 1),
                )
            mu_row = main.tile([1, T], f32, tag="mu", name="mu")
            ms_row = main.tile([1, T], f32, tag="ms", name="ms")
            nc.scalar.mul(mu_row[:], ps_mu, 1.0 / D)
            nc.scalar.mul(ms_row[:], ps_ms, 1.0 / D)
            var_row = main.tile([1, T], f32, tag="var", name="var")
            nc.vector.tensor_tensor(var_row[:], mu_row[:], mu_row[:], OP.mult)
            nc.vector.tensor_sub(var_row[:], ms_row[:], var_row[:])
            sd_row2 = main.tile([1, T], f32, tag="sd", name="sd2")
            nc.scalar.activation(sd_row2[:], var_row[:], AF.Sqrt, bias=eps_ln[:])
            rstd_row = main.tile([1, T], f32, tag="rstd", name="rstd")
            nc.vector.reciprocal(rstd_row[:], sd_row2[:])
            crow_r = main.tile([1, T], f32r, tag="mu2", name="crow_r")
            nc.vector.scalar_tensor_tensor(
                crow_r[:], mu_row[:], -1.0, rstd_row[:], op0=OP.mult, op1=OP.mult
            )
            rstd_r = main.tile([1, T], f32r, tag="ms2", name="rstd_r")
            nc.vector.tensor_copy(rstd_r[:], rstd_row[:])
            bcasts = []
            for row in (rstd_r, crow_r):
                ps_b = psA.tile([P, T], f32, tag="bc", name=f"lnb{len(bcasts)}")
                nc.tensor.matmul(
                    ps_b, lhsT=ones_row_r, rhs=row[:], start=True, stop=True
                )
                bcasts.append(ps_b)
            rstdB, cB = bcasts
            nrm = []
            for dt_i in range(ND):
                nc.vector.tensor_tensor(oT[dt_i][:], oT[dt_i][:], rstdB, OP.mult)
                nc.vector.tensor_tensor(oT[dt_i][:], oT[dt_i][:], cB, OP.add)
                n_ = main.tile([P, T], f32r, tag=f"wkw{dt_i}", name=f"nrm{dt_i}")
                nc.vector.scalar_tensor_tensor(
                    n_[:], oT[dt_i][:], g_sb[:, dt_i : dt_i + 1],
                    b_sb[:, dt_i : dt_i + 1].to_broadcast([P, T]),
                    op0=OP.mult, op1=OP.add,
                )
                nrm.append(n_)

            # ---------- outT[q, t] = Wout @ normed.T + bout ----------
            for qt, (off, sz) in enumerate(QD_TILES):
                ps = psmm.tile([P, T], f32, tag="mm")
                for e in range(ND):
                    nc.tensor.matmul(
                        ps[:sz, :], lhsT=woutT[e][:, ds(off, sz)], rhs=nrm[e][:],
                        start=(e == 0), stop=(e == ND - 1),
                    )
                ot_sb = scr2.tile([P, T], f32, tag="ot")
                nc.scalar.add(ot_sb[:sz, :], ps[:sz, :], bout_sb[:sz, qt : qt + 1])
                nc.sync.dma_start(out_dram.ap()[ds(off, sz), :], ot_sb[:sz, :])

    nc.compile()
    return nc


def _prep_in_maps(inputs):
    def c(a):
        return np.ascontiguousarray(a, dtype=np.float32)

    q = np.asarray(inputs["query_states"], dtype=np.float32).reshape(B * N, QD)
    shared = {
        "WqpT": c(np.asarray(inputs["Wqp"]).T),
        "WqT": c(np.asarray(inputs["Wq"]).T),
        "WkT": c(np.asarray(inputs["Wk"]).T),
        "WvT": c(np.asarray(inputs["Wv"]).T),
        "WoT": c(np.asarray(inputs["Wo"]).T),
        "WoutT": c(np.asarray(inputs["Wout"]).T),
        "memkT": c(np.asarray(inputs["mem_keys"]).T),
        "memvT": c(np.asarray(inputs["mem_values"]).T),
        "ln_g": c(np.asarray(inputs["ln_g"])),
        "ln_b": c(np.asarray(inputs["ln_b"])),
        "bout": c(np.pad(np.asarray(inputs["bout"]), (0, 384 - QD))),
    }
    in_maps = []
    for core in range(NCORES):
        m = dict(shared)
        m["queryT"] = c(q[core * T : (core + 1) * T, :].T)
        in_maps.append(m)
    return in_maps


def kernel(**inputs) -> np.ndarray:
    if "nc" not in _CACHE:
        _CACHE["nc"] = _build_nc()
    nc = _CACHE["nc"]
    in_maps = _prep_in_maps(inputs)
    res = run_bass_kernel_spmd(nc, in_maps, core_ids=list(range(NCORES)))
    out = np.empty((B * N, QD), dtype=np.float32)
    for core in range(NCORES):
        out[core * T : (core + 1) * T, :] = res.results[core]["outT"].T
    return out.reshape(B, N, QD)



# revision 36
# speedup vs baseline: 1.3416x; 1.3416x over previous
"""GatedLTMMemory kernel for 8 Trainium2 NeuronCores.

Data-parallel over the 4096 flattened (B,N) tokens: 512 tokens per core.
Memory-slot tables and weights are replicated. The reference's per-selected-slot
projections are replaced by projecting the slot tables once and running a
masked full-softmax over all S slots (exactly equivalent math).

Schedule notes (engine-time balanced against the TimelineSim cost model):
  - selection path (q projection, key norms, scores, top-32) must be exact
    fp32: reduced precision flips top-32 boundary tokens (gaps ~3.6e-6) and
    blows the output error (measured 0.2 rel err with tf32 scores).
  - f32r matmul operands must be produced by an f32r-writing instruction or
    an f32r DMA (BIR verifier); hence the explicit qTr/ktr rounding copies.
  - keys sum-of-squares on DVE (accum_out), vals on ACT from a bf16 copy.
  - rsqB broadcast stays in PSUM; the key rescale reads it from there.
  - attention: exps merged in pairs on ACT (the bottleneck engine), w-mults
    on DVE except two Pool units per head whose ctx matmuls are deferred,
    ctx matmul flipped token-major (65-row outputs), softmax denominators
    from the appended ones-column of Vp, divided per-token via tensor_scalar.
  - PSUM start=True resets the whole bank: packed-region accumulations are
    pre-zeroed and run with start=False.
  - epilogue token-major: row-wise LayerNorm (ACT accum stats + ln/exp rstd),
    ln_g folded into Wout on device, ln_b folded into the output bias.
  - PE warmup beats the p-state ramp; dummy Sqrt/Ln preload the only two ACT
    tables used, keeping table switches off the critical path.
"""

import numpy as np

import concourse.bacc as bacc
import concourse.mybir as mybir
import concourse.tile as tile
from concourse.bass import ds, ts
from concourse.bass_utils import run_bass_kernel_spmd
from concourse.masks import make_identity

B, N, QD, D, S, H, K = 4, 1024, 320, 512, 1024, 8, 32
DH = D // H
EPS = 1e-5
P = 128
T = 512                       # tokens per core
NCORES = 8
NT = T // P                   # 4 token tiles
ND = D // P                   # 4 contraction chunks over D
NS = S // P                   # 8 slot tiles
QD_TILES = [(0, 128), (128, 128), (256, 64)]

f32 = mybir.dt.float32
f32r = mybir.dt.float32r
bf16 = mybir.dt.bfloat16
AF = mybir.ActivationFunctionType
OP = mybir.AluOpType

_CACHE: dict = {}


def _build_nc():
    nc = bacc.Bacc("TRN2", target_bir_lowering=False, debug=False)

    dr = {}

    def din(name, shape, dt_):
        dr[name] = nc.dram_tensor(name, shape, dt_, kind="ExternalInput")

    din("queryT", (QD, T), f32)
    din("WqpT", (QD, D), f32)
    din("memk_nat", (S, D), f32)
    din("memkT", (D, S), f32)
    din("WkT", (D, D), f32r)
    din("WqT", (D, D), f32r)
    din("WoutT", (D, QD), f32r)
    din("ln_g", (D,), f32)
    din("ln_b", (D,), f32r)
    din("bout", (384,), f32)
    din("memv_nat", (S, D), bf16)
    din("memvT", (D, S), f32r)
    din("WvT", (D, D), f32r)
    din("WoT", (D, D), f32r)
    out_dram = nc.dram_tensor("outT", (QD, T), f32, kind="ExternalOutput")

    with tile.TileContext(nc) as tc:
        with (
            tc.tile_pool(name="const", bufs=1) as const,
            tc.tile_pool(name="main", bufs=1) as main,
            tc.tile_pool(name="sq2", bufs=2) as sq2,
            tc.tile_pool(name="psW", bufs=1, space="PSUM") as psW,
            tc.tile_pool(name="psatt", bufs=2, space="PSUM") as psatt,
            tc.tile_pool(name="psctx", bufs=2, space="PSUM") as psctx,
            nc.allow_low_precision(reason="validated f32r/bf16 paths"),
        ):
            # ---------- DMA issue order == need order (SP configs serially) --
            qryT, wqpT = [], []
            for (off, sz), i in zip(QD_TILES, range(3)):
                t_ = main.tile([sz, T], f32, tag=f"qry{i}", name=f"qry{i}")
                nc.sync.dma_start(t_[:], dr["queryT"].ap()[ds(off, sz), :])
                qryT.append(t_)
            for (off, sz), i in zip(QD_TILES, range(3)):
                t_ = main.tile([sz, D], f32, tag=f"wqp{i}", name=f"wqp{i}")
                nc.sync.dma_start(t_[:], dr["WqpT"].ap()[ds(off, sz), :])
                wqpT.append(t_)
            # keys slot-major (for exact ssq), two DMAs of 4 slot-tiles each
            knat = []
            for hhalf in range(2):
                t_ = main.tile([P, 4, D], f32, tag=f"kn{hhalf}", name=f"kn{hhalf}")
                nc.sync.dma_start(
                    t_[:],
                    dr["memk_nat"].ap()[ds(hhalf * 4 * P, 4 * P), :]
                    .rearrange("(o p) d -> p o d", p=P),
                )
                knat.append(t_)
            # keys feature-major [d, s] (scores + source of ktr)
            ktb = main.tile([P, ND, S], f32, tag="ktb", name="ktb")
            nc.sync.dma_start(
                ktb[:], dr["memkT"].ap().rearrange("(o p) s -> p o s", p=P)
            )
            keysT = [ktb[:, dc, :] for dc in range(ND)]
            wkb = main.tile([P, ND, D], f32r, tag="wkb", name="wkb")
            nc.sync.dma_start(
                wkb[:], dr["WkT"].ap().rearrange("(o p) d -> p o d", p=P)
            )
            wkT = [wkb[:, dc, :] for dc in range(ND)]
            wqb = main.tile([P, ND, D], f32r, tag="wqb", name="wqb")
            nc.sync.dma_start(
                wqb[:], dr["WqT"].ap().rearrange("(o p) d -> p o d", p=P)
            )
            wqT = [wqb[:, dc, :] for dc in range(ND)]
            woutb = main.tile([P, ND, QD], f32r, tag="woutb", name="woutb")
            nc.sync.dma_start(
                woutb[:], dr["WoutT"].ap().rearrange("(o p) q -> p o q", p=P)
            )
            woutT = [woutb[:, e, :] for e in range(ND)]
            g_sb = const.tile([P, ND], f32, tag="g")
            nc.sync.dma_start(g_sb[:], dr["ln_g"].ap().rearrange("(o p) -> p o", p=P))
            b_sb = const.tile([P, ND], f32r, tag="b")
            nc.sync.dma_start(b_sb[:], dr["ln_b"].ap().rearrange("(o p) -> p o", p=P))
            bout_row = const.tile([1, 384], f32, tag="bout")
            nc.sync.dma_start(
                bout_row[:], dr["bout"].ap().rearrange("(o q) -> o q", o=1)
            )
            # vals slot-major in bf16 (loose ssq), one DMA
            vnb = main.tile([P, NS, D], bf16, tag="vnb", name="vnb")
            nc.sync.dma_start(
                vnb[:], dr["memv_nat"].ap().rearrange("(o p) d -> p o d", p=P)
            )
            vtb = main.tile([P, ND, S], f32r, tag="vtb", name="vtb")
            nc.sync.dma_start(
                vtb[:], dr["memvT"].ap().rearrange("(o p) s -> p o s", p=P)
            )
            valsT = [vtb[:, dc, :] for dc in range(ND)]
            wvb = main.tile([P, ND, D], f32r, tag="wvb", name="wvb")
            nc.sync.dma_start(
                wvb[:], dr["WvT"].ap().rearrange("(o p) d -> p o d", p=P)
            )
            wvT = [wvb[:, dc, :] for dc in range(ND)]
            wob = main.tile([P, ND, D], f32r, tag="wob", name="wob")
            nc.sync.dma_start(
                wob[:], dr["WoT"].ap().rearrange("(o p) d -> p o d", p=P)
            )
            woT = [wob[:, dc, :] for dc in range(ND)]

            # ---------- constants ----------
            ident = const.tile([P, P], bf16, tag="ident")
            make_identity(nc, ident)
            ident_f = const.tile([P, P], f32, tag="ident_f")
            make_identity(nc, ident_f)
            eps12 = const.tile([P, 1], f32, tag="eps12")
            nc.vector.memset(eps12, 1e-12)
            eps_ln = const.tile([P, 1], f32, tag="eps_ln")
            nc.vector.memset(eps_ln, EPS)
            ones_row = const.tile([1, P], f32, tag="ones_row")
            nc.vector.memset(ones_row, 1.0)
            onesT_f = const.tile([1, T], f32, tag="onesT_f")
            nc.vector.memset(onesT_f, 1.0)
            ones_rT = const.tile([1, T], f32r, tag="ones_rT")
            nc.vector.tensor_copy(ones_rT[:], onesT_f[:])
            dummy = const.tile([1, 1], f32, tag="dummy")
            nc.vector.memset(dummy, 1.0)

            # small stat columns
            ssq_k = const.tile([P, NS], f32, tag="ssq_k")
            rsq_k = const.tile([P, NS], f32, tag="rsq_k")
            ssq_v = const.tile([P, NS], f32, tag="ssq_v")
            rsq_v = const.tile([P, NS], f32, tag="rsq_v")
            brow = const.tile([1, 384], f32r, tag="brow")
            mu_c = const.tile([P, NT], f32, tag="mu_c")
            ssq_c = const.tile([P, NT], f32, tag="ssq_c")
            mu2_c = const.tile([P, NT], f32, tag="mu2_c")
            var_c = const.tile([P, NT], f32, tag="var_c")
            rstd_c = const.tile([P, NT], f32, tag="rstd_c")
            bout2 = const.tile([P, 3], f32, tag="bout2")
            rsq_row = main.tile([1, S], f32, tag="tkwork", name="rsq_row")

            # ---------- ACT: preload the sqrt table before anything else -----
            nc.scalar.activation(dummy[:], dummy[:], AF.Sqrt)

            # ---------- PE warmup: beat the p-state ramp during DMA wait -----
            for i in range(34):
                wps = psW.tile(
                    [P, T], f32, tag="mmA" if i % 2 == 0 else "mmB", name=f"wu{i}"
                )
                nc.tensor.matmul(
                    wps[:, 0:P], lhsT=ident, rhs=ident, start=True, stop=True
                )

            # ---------- keys ssq on DVE (exact), vp ones-columns on Pool -----
            for j in range(NS):
                scr = sq2.tile([P, D], f32, tag="sq")
                nc.vector.scalar_tensor_tensor(
                    scr[:], knat[j // 4][:, j % 4, :], 1.0,
                    knat[j // 4][:, j % 4, :],
                    op0=OP.mult, op1=OP.mult,
                    accum_out=ssq_k[:, j : j + 1],
                )
            vp = []
            for st in range(NS):
                t_ = main.tile([P, H, DH + 1], bf16, tag=f"vp{st}", name=f"vp{st}")
                nc.gpsimd.memset(t_[:, :, DH : DH + 1], 1.0)
                vp.append(t_)

            # ---------- qT[d, t] = Wqp @ query.T (exact fp32) ----------
            qT = []
            for dt_i in range(ND):
                t_ = main.tile([P, T], f32, tag=f"qt{dt_i}", name=f"q{dt_i}")
                ps = psW.tile(
                    [P, T], f32, tag="mmA" if dt_i % 2 == 0 else "mmB",
                    name=f"psq{dt_i}",
                )
                for c in range(3):
                    nc.tensor.matmul(
                        ps, lhsT=wqpT[c][:, ts(dt_i, P)], rhs=qryT[c][:],
                        start=(c == 0), stop=(c == 2),
                    )
                if dt_i == 2:
                    # rsq transposes slot in here (emitted below, PE order)
                    pass
                nc.scalar.copy(t_[:], ps)
                qT.append(t_)
                if dt_i == 2:
                    nc.scalar.activation(
                        rsq_k[:], ssq_k[:], AF.Sqrt, bias=eps12[:]
                    )
            nc.vector.reciprocal(rsq_k[:], rsq_k[:])

            # rsq_k columns -> one [1, S] row (PE transposes into psatt banks)
            ps_rowA = psatt.tile([1, T], f32, tag="att", name="ps_rowA")
            for j in range(4):
                nc.tensor.matmul(
                    ps_rowA[0:1, ts(j, P)], lhsT=rsq_k[:, j : j + 1], rhs=ident_f,
                    is_transpose=True, skip_group_check=True,
                )
            ps_rowB = psatt.tile([1, T], f32, tag="att", name="ps_rowB")
            for j in range(4):
                nc.tensor.matmul(
                    ps_rowB[0:1, ts(j, P)], lhsT=rsq_k[:, 4 + j : 5 + j],
                    rhs=ident_f, is_transpose=True, skip_group_check=True,
                )
            nc.vector.tensor_copy(rsq_row[0:1, 0:T], ps_rowA)
            nc.vector.tensor_copy(rsq_row[0:1, T : 2 * T], ps_rowB)
            # rsqB broadcast lives in PSUM only; the rescale reads it there
            ps_rs = psatt.tile([P, 2, T], f32, tag="att", name="ps_rs")
            for half in range(2):
                nc.tensor.matmul(
                    ps_rs[:, half, :], lhsT=ones_row,
                    rhs=rsq_row[0:1, ds(half * T, T)],
                    start=True, stop=True, skip_group_check=True,
                )
            # keys l2-normalized in place (exact; feeds scores), per half so
            # the first score matmul isn't gated on the second half
            for dc in range(ND):
                for half in range(2):
                    nc.vector.tensor_tensor(
                        ktb[:, dc, ds(half * T, T)], ktb[:, dc, ds(half * T, T)],
                        ps_rs[:, half, :], OP.mult,
                    )
            # rounded f32r copies for qhT (DVE, idle window)
            qTr_tags = ["qry0", "qry1", "qry2", "wqp0"]
            qTr = []
            for dt_i in range(ND):
                t_ = main.tile([P, T], f32r, tag=qTr_tags[dt_i], name=f"qr{dt_i}")
                nc.vector.tensor_copy(t_[:], qT[dt_i][:])
                qTr.append(t_)
            # rounded f32r copies of normalized keys for KpT (Pool, off-path)
            ktr_tags = ["kn0", "kn1", "wqp1", "wqp2"]
            ktr = []
            for dc in range(ND):
                t_ = main.tile([P, S], f32r, tag=ktr_tags[dc], name=f"ktr{dc}")
                nc.gpsimd.tensor_copy(t_[:], keysT[dc])
                ktr.append(t_)

            # ---------- vals ssq (ACT, early; bf16 copy is enough) ----------
            for j in range(NS):
                scr = sq2.tile([P, D], f32, tag="sq")
                nc.scalar.activation(
                    scr[:], vnb[:, j, :], AF.Square, accum_out=ssq_v[:, j : j + 1]
                )
            nc.scalar.activation(rsq_v[:], ssq_v[:], AF.Sqrt, bias=eps12[:])
            nc.vector.reciprocal(rsq_v[:], rsq_v[:])
            # preload the ln/exp table set (all later ACT funcs live in it)
            nc.scalar.activation(dummy[:], dummy[:], AF.Ln)

            # ---------- scores[t, s] = q @ keysn.T (exact fp32) --------------
            sc = []
            for tt in range(NT):
                t_ = main.tile([P, S], f32, tag=f"sc{tt}", name=f"sc{tt}")
                for half in range(2):
                    ps = psW.tile(
                        [P, T], f32, tag="mmA" if half == 0 else "mmB",
                        name=f"pssc{tt}_{half}",
                    )
                    for dc in range(ND):
                        nc.tensor.matmul(
                            ps,
                            lhsT=qT[dc][:, ts(tt, P)],
                            rhs=keysT[dc][ds(0, P), ds(half * T, T)],
                            start=(dc == 0), stop=(dc == ND - 1),
                        )
                    nc.scalar.copy(t_[:, ds(half * T, T)], ps)
                sc.append(t_)

            # ---------- top-32 threshold per token row (DVE), mask on Pool ---
            mask01 = []
            for tt in range(NT):
                work = main.tile([P, S], f32, tag="tkwork", name=f"wk{tt}")
                cur = sc[tt]
                for r in range(4):
                    mx = const.tile([P, 8], f32, tag=f"mx{tt}_{r}")
                    nc.vector.max(out=mx[:], in_=cur[:])
                    if r < 3:
                        nc.vector.match_replace(
                            out=work[:], in_to_replace=mx[:], in_values=cur[:],
                            imm_value=-1e30,
                        )
                        cur = work
                m_ = main.tile([P, S], bf16, tag=f"mk{tt}", name=f"mk{tt}")
                nc.gpsimd.tensor_scalar(
                    m_[:], sc[tt][:], mx[:, 7:8], None, op0=OP.is_ge
                )
                mask01.append(m_)

            # ---------- KpT[e, s] = Wk @ keysn.T  (f32r) ----------
            kpT = []
            for e in range(ND):
                t_ = main.tile([P, S], f32r, tag=f"kp{e}", name=f"kp{e}")
                for half in range(2):
                    ps = psW.tile(
                        [P, T], f32, tag="mmA" if half == 0 else "mmB",
                        name=f"pskp{e}_{half}",
                    )
                    for dc in range(ND):
                        nc.tensor.matmul(
                            ps,
                            lhsT=wkT[dc][:, ts(e, P)],
                            rhs=ktr[dc][:, ds(half * T, T)],
                            start=(dc == 0), stop=(dc == ND - 1),
                        )
                    nc.scalar.copy(t_[:, ds(half * T, T)], ps)
                kpT.append(t_)

            # ---------- brow[q] = bout + Wout @ ln_b (row layout) ----------
            ps_bc = psW.tile([1, QD], f32, tag="mmA")
            for e in range(ND):
                nc.tensor.matmul(
                    ps_bc, lhsT=b_sb[:, e : e + 1], rhs=woutT[e],
                    start=(e == 0), stop=(e == ND - 1),
                )
            nc.vector.tensor_tensor(
                brow[0:1, 0:QD], bout_row[0:1, 0:QD], ps_bc, OP.add
            )

            # ---------- qhT[e, t] = (Wq @ qT) / 8  (f32r) ----------
            qhT = []
            for e in range(ND):
                t_ = main.tile([P, T], f32r, tag=f"qh{e}", name=f"qh{e}")
                ps = psW.tile(
                    [P, T], f32, tag="mmA" if e % 2 == 0 else "mmB",
                    name=f"psqh{e}",
                )
                for dc in range(ND):
                    nc.tensor.matmul(
                        ps, lhsT=wqT[dc][:, ts(e, P)], rhs=qTr[dc][:],
                        start=(dc == 0), stop=(dc == ND - 1),
                    )
                nc.scalar.mul(t_[:], ps, 1.0 / np.sqrt(DH))
                qhT.append(t_)

            # ---------- Vp[s, 8 heads x (64 + ones)] = valsn @ Wv.T (bf16) ---
            # raw vals in the matmul; the per-slot 1/||v|| lands on the copy.
            def emit_vp(st):
                ps = psW.tile(
                    [P, D], f32, tag="mmA" if st % 2 == 0 else "mmB",
                    name=f"psvp{st}",
                )
                for dc in range(ND):
                    nc.tensor.matmul(
                        ps,
                        lhsT=valsT[dc][:, ts(st, P)],
                        rhs=wvT[dc][:],
                        start=(dc == 0), stop=(dc == ND - 1),
                    )
                nc.scalar.activation(
                    vp[st][:, :, 0:DH],
                    ps.rearrange("p (h e) -> p h e", h=H),
                    AF.Copy, scale=rsq_v[:, st : st + 1],
                )

            emit_vp(0)
            emit_vp(1)

            # ---------- mask transpose to [s, t] (bf16): j0-3 DVE, j4-7 ACT --
            mTh = [
                main.tile([P, 4, T], bf16, tag=f"sc{j}", name=f"mTh{j}")
                for j in range(2)
            ]

            def mT(c):
                return mTh[c // 4][:, c % 4, :]

            for j in range(NS):
                ps_t = psW.tile(
                    [P, T], bf16, tag="mmA" if j % 2 == 0 else "mmB",
                    name=f"psmT{j}",
                )
                for tt in range(NT):
                    nc.tensor.matmul(
                        ps_t[:, ts(tt, P)], lhsT=mask01[tt][:, ts(j, P)],
                        rhs=ident, is_transpose=True, skip_group_check=True,
                    )
                if j < 4:
                    nc.vector.tensor_copy(mT(j), ps_t)
                else:
                    nc.scalar.copy(mT(j), ps_t)

            # ---------- attention: masked softmax over all S slots -----------
            ctx_tok = main.tile([P, NT, T], bf16, tag="wkb", name="cxb")
            ctxT_big = main.tile([P, ND, T], f32r, tag="ctxT", name="ctxT")
            # u ring: 2 pair-slots in uA, 1 in uB; w ring: 5 slots in wbig
            uA = main.tile([P, 4, T], bf16, tag="kn0", name="uA")
            uB = main.tile([P, 2, T], bf16, tag="kn1", name="uB")
            # slots 0-4: DVE w ring; slots 5,6: deferred Pool units (c=1, c=5)
            wbig = main.tile([P, 7, T], bf16, tag="vnb", name="wbig")

            def u_slot(i):
                m = i % 3
                if m < 2:
                    return uA[:, 2 * m : 2 * m + 2, :]
                return uB[:, 0:2, :]

            pairs = [(h, cp) for h in range(H) for cp in range(4)]
            u_of = {}
            ps_ctx_of = {}
            deferred = {}
            widx = [0]

            def att_logits(i):
                h, cp = pairs[i]
                if cp == 0:
                    ps_ctx_of[h] = psctx.tile(
                        [P, NT, DH + 1], f32, tag="ctx", name=f"ctx{h}"
                    )
                    nc.vector.memset(ps_ctx_of[h][:], 0.0)
                    deferred[h] = []
                e, ro = h // 2, (h % 2) * DH
                ps = psatt.tile([P, 2, T], f32, tag="att", name=f"att{h}_{cp}")
                for j in range(2):
                    c = 2 * cp + j
                    nc.tensor.matmul(
                        ps[:, j, :],
                        lhsT=kpT[e][ro : ro + DH, ts(c, P)],
                        rhs=qhT[e][ro : ro + DH, :],
                        start=True, stop=True, skip_group_check=True,
                    )
                u2 = u_slot(i)
                nc.scalar.activation(u2, ps[:], AF.Exp)
                u_of[i] = u2

            def ctx_mms(h, c, w, stop):
                ps_ctx = ps_ctx_of[h]
                for tt in range(NT):
                    nc.tensor.matmul(
                        ps_ctx[:, tt, :],
                        lhsT=w[:, ts(tt, P)],
                        rhs=vp[c][:, h, :],
                        start=False, stop=stop,
                        skip_group_check=True,
                    )

            def att_wctx(i):
                h, cp = pairs[i]
                u2 = u_of.pop(i)
                for j in range(2):
                    c = 2 * cp + j
                    if c in (1, 5):
                        # Pool unit: slow mult, ctx matmuls deferred to the
                        # end of the head so the in-order PE never waits on it
                        w = wbig[:, 5 + (c == 5), :]
                        nc.gpsimd.tensor_tensor(w, u2[:, j, :], mT(c), OP.mult)
                        deferred[h].append((c, w))
                    else:
                        w = wbig[:, widx[0] % 5, :]
                        widx[0] += 1
                        nc.vector.tensor_tensor(w, u2[:, j, :], mT(c), OP.mult)
                        ctx_mms(h, c, w, stop=False)
                if cp == 3:
                    for idx, (c, w) in enumerate(deferred[h]):
                        ctx_mms(h, c, w, stop=(idx == len(deferred[h]) - 1))
                    drain_head(h)
                    if h % 2 == 1:
                        wo_pair(h // 2)

            def drain_head(h):
                ps_ctx = ps_ctx_of.pop(h)
                rcp = const.tile([P, NT], f32, tag=f"rcp{h % 2}", name=f"rcp{h}")
                nc.vector.reciprocal(rcp[:], ps_ctx[:, :, DH : DH + 1])
                for tt in range(NT):
                    nc.vector.tensor_scalar(
                        ctx_tok[:, tt, ds(h * DH, DH)],
                        ps_ctx[:, tt, 0:DH],
                        rcp[:, tt : tt + 1], None, op0=OP.mult,
                    )

            def wo_pair(e):
                # transpose ctx cols for heads (2e, 2e+1) into ctxT_big[:, e, :]
                ps_t = psW.tile(
                    [P, T], bf16, tag="mmA" if e % 2 == 0 else "mmB", name=f"pt{e}"
                )
                for tt in range(NT):
                    nc.tensor.matmul(
                        ps_t[:, ts(tt, P)], lhsT=ctx_tok[:, tt, ts(e, P)],
                        rhs=ident, is_transpose=True, skip_group_check=True,
                    )
                nc.vector.tensor_copy(ctxT_big[:, e, :], ps_t)

            # software-pipelined: logits/exp of pair i overlap w+ctx of i-1;
            # the first h0 pairs are zippered with the remaining Vp tiles.
            att_logits(0)
            emit_vp(2)
            emit_vp(3)
            att_logits(1)
            att_wctx(0)
            emit_vp(4)
            emit_vp(5)
            att_logits(2)
            att_wctx(1)
            emit_vp(6)
            emit_vp(7)
            for i in range(3, len(pairs)):
                att_logits(i)
                att_wctx(i - 1)
            att_wctx(len(pairs) - 1)

            # fold ln_g into Wout rows (Pool; WoutT bconst reads are done)
            for e in range(ND):
                nc.gpsimd.tensor_scalar(
                    woutT[e], woutT[e], g_sb[:, e : e + 1], None, op0=OP.mult
                )

            # ---------- o[t, e] = ctx.T @ Wo.T (token-major), LN row-wise ----
            o_big = main.tile([P, NT, D], f32, tag="ktb", name="o_big")
            for tt in range(NT):
                ps_o = psW.tile(
                    [P, D], f32, tag="mmA" if tt % 2 == 0 else "mmB",
                    name=f"pso{tt}",
                )
                for dc in range(ND):
                    nc.tensor.matmul(
                        ps_o,
                        lhsT=ctxT_big[:, dc, ts(tt, P)],
                        rhs=woT[dc][:],
                        start=(dc == 0), stop=(dc == ND - 1),
                    )
                nc.scalar.activation(
                    o_big[:, tt, :], ps_o, AF.Copy, accum_out=mu_c[:, tt : tt + 1]
                )
                scr = sq2.tile([P, D], f32, tag="sq")
                nc.scalar.activation(
                    scr[:], o_big[:, tt, :], AF.Square,
                    accum_out=ssq_c[:, tt : tt + 1],
                )

            # var = ssq/D - mu^2; rstd = exp(-0.5 * ln(var + eps))
            nc.vector.tensor_scalar(mu2_c[:], mu_c[:], 1.0 / D, None, op0=OP.mult)
            nc.vector.tensor_tensor(var_c[:], mu2_c[:], mu2_c[:], OP.mult)
            nc.vector.scalar_tensor_tensor(
                var_c[:], ssq_c[:], 1.0 / D, var_c[:], op0=OP.mult, op1=OP.subtract
            )
            nc.scalar.activation(rstd_c[:], var_c[:], AF.Ln, bias=eps_ln[:])
            nc.scalar.activation(var_c[:], rstd_c[:], AF.Exp, scale=-0.5)
            rstd_c = var_c

            # xhat = (o - mu) * rstd  (bf16, token-major), then transpose
            xh_big = main.tile([P, NT, D], bf16, tag="vtb", name="xh_big")
            for tt in range(NT):
                nc.vector.scalar_tensor_tensor(
                    xh_big[:, tt, :], o_big[:, tt, :], mu2_c[:, tt : tt + 1],
                    rstd_c[:, tt : tt + 1].to_broadcast([P, D]),
                    op0=OP.subtract, op1=OP.mult,
                )
            xhatT_big = main.tile([P, ND, T], f32r, tag="xhT", name="xhT")
            for e in range(ND):
                ps_t = psW.tile(
                    [P, T], bf16, tag="mmA" if e % 2 == 0 else "mmB",
                    name=f"psxT{e}",
                )
                for tt in range(NT):
                    nc.tensor.matmul(
                        ps_t[:, ts(tt, P)], lhsT=xh_big[:, tt, ts(e, P)],
                        rhs=ident, is_transpose=True, skip_group_check=True,
                    )
                if e % 2 == 0:
                    nc.vector.tensor_copy(xhatT_big[:, e, :], ps_t)
                else:
                    nc.scalar.copy(xhatT_big[:, e, :], ps_t)

            # ---------- outT[q, t] = (g*Wout).T @ xhatT + brow ----------
            for qt, (off, sz) in enumerate(QD_TILES):
                ps = psW.tile(
                    [P, T], f32, tag="mmA" if qt % 2 == 0 else "mmB",
                    name=f"psout{qt}",
                )
                for e in range(ND):
                    nc.tensor.matmul(
                        ps[:sz, :], lhsT=woutT[e][:, ds(off, sz)],
                        rhs=xhatT_big[:, e, :],
                        start=(e == 0), stop=False,
                    )
                nc.tensor.matmul(
                    ps[:sz, :], lhsT=brow[0:1, ds(off, sz)], rhs=ones_rT[:],
                    start=False, stop=True,
                )
                ot_sb = sq2.tile([P, T], f32, tag="sq", name=f"ot{qt}")
                nc.scalar.copy(ot_sb[:sz, :], ps[:sz, :])
                nc.sync.dma_start(out_dram.ap()[ds(off, sz), :], ot_sb[:sz, :])

    nc.compile()
    return nc


def _prep_in_maps(inputs):
    import ml_dtypes

    def c(a):
        return np.ascontiguousarray(a, dtype=np.float32)

    q = np.asarray(inputs["query_states"], dtype=np.float32).reshape(B * N, QD)
    vals = np.asarray(inputs["mem_values"], dtype=np.float32)
    shared = {
        "WqpT": c(np.asarray(inputs["Wqp"]).T),
        "WqT": c(np.asarray(inputs["Wq"]).T),
        "WkT": c(np.asarray(inputs["Wk"]).T),
        "WvT": c(np.asarray(inputs["Wv"]).T),
        "WoT": c(np.asarray(inputs["Wo"]).T),
        "WoutT": c(np.asarray(inputs["Wout"]).T),
        "memk_nat": c(np.asarray(inputs["mem_keys"])),
        "memkT": c(np.asarray(inputs["mem_keys"]).T),
        "memv_nat": np.ascontiguousarray(vals.astype(ml_dtypes.bfloat16)),
        "memvT": c(vals.T),
        "ln_g": c(np.asarray(inputs["ln_g"])),
        "ln_b": c(np.asarray(inputs["ln_b"])),
        "bout": c(np.pad(np.asarray(inputs["bout"]), (0, 384 - QD))),
    }
    in_maps = []
    for core in range(NCORES):
        m = dict(shared)
        m["queryT"] = c(q[core * T : (core + 1) * T, :].T)
        in_maps.append(m)
    return in_maps


def kernel(**inputs) -> np.ndarray:
    if "nc" not in _CACHE:
        _CACHE["nc"] = _build_nc()
    nc = _CACHE["nc"]
    in_maps = _prep_in_maps(inputs)
    res = run_bass_kernel_spmd(nc, in_maps, core_ids=list(range(NCORES)))
    out = np.empty((B * N, QD), dtype=np.float32)
    for core in range(NCORES):
        out[core * T : (core + 1) * T, :] = res.results[core]["outT"].T
    return out.reshape(B, N, QD)


# revision 54
# speedup vs baseline: 1.3895x; 1.0357x over previous
"""GatedLTMMemory kernel for 8 Trainium2 NeuronCores.

Data-parallel over the 4096 flattened (B,N) tokens: 512 tokens per core.
Memory-slot tables and weights are replicated. The reference's per-selected-slot
projections are replaced by projecting the slot tables once and running a
masked full-softmax over all S slots (exactly equivalent math).

Schedule notes (engine-time balanced against the TimelineSim cost model):
  - selection path (q projection, key norms, scores, top-32) must be exact
    fp32: reduced precision flips top-32 boundary tokens (gaps ~3.6e-6) and
    blows the output error (measured 0.2 rel err with tf32 scores).
  - f32r matmul operands must be produced by an f32r-writing instruction or
    an f32r DMA (BIR verifier); hence the explicit qTr/ktr rounding copies.
  - keys sum-of-squares on DVE (accum_out), vals on ACT from a bf16 copy.
  - rsqB broadcast stays in PSUM; the key rescale reads it from there.
  - attention: exps merged in pairs on ACT (the bottleneck engine), w-mults
    on DVE except two Pool units per head whose ctx matmuls are deferred,
    ctx matmul flipped token-major (65-row outputs), softmax denominators
    from the appended ones-column of Vp, divided per-token via tensor_scalar.
  - PSUM start=True resets the whole bank: packed-region accumulations are
    pre-zeroed and run with start=False.
  - epilogue token-major: row-wise LayerNorm (ACT accum stats + ln/exp rstd),
    ln_g folded into Wout on device, ln_b folded into the output bias.
  - PE warmup beats the p-state ramp; dummy Sqrt/Exp activations preload the
    ACT tables, keeping table switches off the critical path.
  - DMA transfers execute serially in issue order (~43us total), so the DMA
    program is ordered by first-use time.
"""

import numpy as np

import concourse.bacc as bacc
import concourse.mybir as mybir
import concourse.tile as tile
from concourse.bass import ds, ts
from concourse.bass_utils import run_bass_kernel_spmd
from concourse.masks import make_identity

B, N, QD, D, S, H, K = 4, 1024, 320, 512, 1024, 8, 32
DH = D // H
EPS = 1e-5
P = 128
T = 512                       # tokens per core
NCORES = 8
NT = T // P                   # 4 token tiles
ND = D // P                   # 4 contraction chunks over D
NS = S // P                   # 8 slot tiles
QD_TILES = [(0, 128), (128, 128), (256, 64)]

f32 = mybir.dt.float32
f32r = mybir.dt.float32r
bf16 = mybir.dt.bfloat16
AF = mybir.ActivationFunctionType
OP = mybir.AluOpType

_CACHE: dict = {}


def _build_nc():
    nc = bacc.Bacc("TRN2", target_bir_lowering=False, debug=False)

    dr = {}

    def din(name, shape, dt_):
        dr[name] = nc.dram_tensor(name, shape, dt_, kind="ExternalInput")

    din("queryT", (QD, T), f32)
    din("WqpT", (QD, D), f32)
    din("memk_nat", (S, D), f32)
    din("memkT", (D, S), f32)
    din("WkT", (D, D), f32r)
    din("WqT", (D, D), f32r)
    din("WoutT", (D, QD), f32r)
    din("ln_g", (D,), f32)
    din("ln_b", (D,), f32r)
    din("bout", (384,), f32)
    din("memv_nat", (S, D), bf16)
    din("memvT", (D, S), f32r)
    din("WvT", (D, D), f32r)
    din("WoT", (D, D), f32r)
    out_dram = nc.dram_tensor("outT", (QD, T), f32, kind="ExternalOutput")

    with tile.TileContext(nc) as tc:
        with (
            tc.tile_pool(name="const", bufs=1) as const,
            tc.tile_pool(name="main", bufs=1) as main,
            tc.tile_pool(name="sq2", bufs=2) as sq2,
            tc.tile_pool(name="psW", bufs=1, space="PSUM") as psW,
            tc.tile_pool(name="psatt", bufs=4, space="PSUM") as psatt,
            tc.tile_pool(name="psctx", bufs=2, space="PSUM") as psctx,
            nc.allow_low_precision(reason="validated f32r/bf16 paths"),
        ):
            # ---------- DMA issue order == need order (SP configs serially) --
            # qry/wqp chunk-interleaved so qT matmul c starts as chunk c lands
            qryT, wqpT = [], []
            for (off, sz), i in zip(QD_TILES, range(3)):
                tq = main.tile([sz, T], f32, tag=f"qry{i}", name=f"qry{i}")
                nc.sync.dma_start(tq[:], dr["queryT"].ap()[ds(off, sz), :])
                qryT.append(tq)
                tw = main.tile([sz, D], f32, tag=f"wqp{i}", name=f"wqp{i}")
                nc.sync.dma_start(tw[:], dr["WqpT"].ap()[ds(off, sz), :])
                wqpT.append(tw)
            # keys slot-major (ssq chain); DMA transfers are serial in issue
            # order, so these follow the qT inputs
            knat = []
            for hhalf in range(2):
                t_ = main.tile([P, 4, D], f32, tag=f"kn{hhalf}", name=f"kn{hhalf}")
                nc.sync.dma_start(
                    t_[:],
                    dr["memk_nat"].ap()[ds(hhalf * 4 * P, 4 * P), :]
                    .rearrange("(o p) d -> p o d", p=P),
                )
                knat.append(t_)
            # keys feature-major [d, s] (scores + source of ktr)
            ktb = main.tile([P, ND, S], f32, tag="ktb", name="ktb")
            nc.sync.dma_start(
                ktb[:], dr["memkT"].ap().rearrange("(o p) s -> p o s", p=P)
            )
            keysT = [ktb[:, dc, :] for dc in range(ND)]
            wkb = main.tile([P, ND, D], f32r, tag="wkb", name="wkb")
            nc.sync.dma_start(
                wkb[:], dr["WkT"].ap().rearrange("(o p) d -> p o d", p=P)
            )
            wkT = [wkb[:, dc, :] for dc in range(ND)]
            wqb = main.tile([P, ND, D], f32r, tag="wqb", name="wqb")
            nc.sync.dma_start(
                wqb[:], dr["WqT"].ap().rearrange("(o p) d -> p o d", p=P)
            )
            wqT = [wqb[:, dc, :] for dc in range(ND)]
            woutb = main.tile([P, ND, QD], f32r, tag="woutb", name="woutb")
            nc.sync.dma_start(
                woutb[:], dr["WoutT"].ap().rearrange("(o p) q -> p o q", p=P)
            )
            woutT = [woutb[:, e, :] for e in range(ND)]
            g_sb = const.tile([P, ND], f32, tag="g")
            nc.sync.dma_start(g_sb[:], dr["ln_g"].ap().rearrange("(o p) -> p o", p=P))
            b_sb = const.tile([P, ND], f32r, tag="b")
            nc.sync.dma_start(b_sb[:], dr["ln_b"].ap().rearrange("(o p) -> p o", p=P))
            bout_row = const.tile([1, 384], f32, tag="bout")
            nc.sync.dma_start(
                bout_row[:], dr["bout"].ap().rearrange("(o q) -> o q", o=1)
            )
            # vals slot-major in bf16 (loose ssq), one DMA
            vnb = main.tile([P, NS, D], bf16, tag="vnb", name="vnb")
            nc.sync.dma_start(
                vnb[:], dr["memv_nat"].ap().rearrange("(o p) d -> p o d", p=P)
            )
            vtb = main.tile([P, ND, S], f32r, tag="vtb", name="vtb")
            nc.sync.dma_start(
                vtb[:], dr["memvT"].ap().rearrange("(o p) s -> p o s", p=P)
            )
            valsT = [vtb[:, dc, :] for dc in range(ND)]
            wvb = main.tile([P, ND, D], f32r, tag="wvb", name="wvb")
            nc.sync.dma_start(
                wvb[:], dr["WvT"].ap().rearrange("(o p) d -> p o d", p=P)
            )
            wvT = [wvb[:, dc, :] for dc in range(ND)]
            wob = main.tile([P, ND, D], f32r, tag="wob", name="wob")
            nc.sync.dma_start(
                wob[:], dr["WoT"].ap().rearrange("(o p) d -> p o d", p=P)
            )
            woT = [wob[:, dc, :] for dc in range(ND)]

            # ---------- constants ----------
            ident = const.tile([P, P], bf16, tag="ident")
            make_identity(nc, ident)
            ident_f = const.tile([P, P], f32, tag="ident_f")
            make_identity(nc, ident_f)
            eps12 = const.tile([P, 1], f32, tag="eps12")
            nc.vector.memset(eps12, 1e-12)
            eps_ln = const.tile([P, 1], f32, tag="eps_ln")
            nc.vector.memset(eps_ln, EPS)
            ones_row = const.tile([1, P], f32, tag="ones_row")
            nc.vector.memset(ones_row, 1.0)
            onesT_f = const.tile([1, T], f32, tag="onesT_f")
            nc.vector.memset(onesT_f, 1.0)
            ones_rT = const.tile([1, T], f32r, tag="ones_rT")
            nc.vector.tensor_copy(ones_rT[:], onesT_f[:])
            dummy = const.tile([1, 1], f32, tag="dummy")
            nc.vector.memset(dummy, 1.0)
            zt = const.tile([P, P], bf16, tag="zt")
            nc.vector.memset(zt, 0.0)

            # small stat columns
            ssq_k = const.tile([P, NS], f32, tag="ssq_k")
            rsq_k = const.tile([P, NS], f32, tag="rsq_k")
            ssq_v = const.tile([P, NS], f32, tag="ssq_v")
            rsq_v = const.tile([P, NS], f32, tag="rsq_v")
            brow = const.tile([1, 384], f32r, tag="brow")
            mu_c = const.tile([P, NT], f32, tag="mu_c")
            ssq_c = const.tile([P, NT], f32, tag="ssq_c")
            mu2_c = const.tile([P, NT], f32, tag="mu2_c")
            var_c = const.tile([P, NT], f32, tag="var_c")
            rstd_c = const.tile([P, NT], f32, tag="rstd_c")
            bout2 = const.tile([P, 3], f32, tag="bout2")
            rsq_row = main.tile([1, S], f32, tag="tkwork", name="rsq_row")

            # ---------- ACT: preload the sqrt table before anything else -----
            nc.scalar.activation(dummy[:], dummy[:], AF.Sqrt)

            # ---------- PE warmup: beat the p-state ramp during DMA wait -----
            for i in range(34):
                wps = psW.tile(
                    [P, T], f32, tag="mmA" if i % 2 == 0 else "mmB", name=f"wu{i}"
                )
                nc.tensor.matmul(
                    wps[:, 0:P], lhsT=ident, rhs=ident, start=True, stop=True
                )

            # ---------- keys ssq on DVE (exact), vp ones-columns on Pool -----
            for j in range(NS):
                scr = sq2.tile([P, D], f32, tag="sq")
                nc.vector.scalar_tensor_tensor(
                    scr[:], knat[j // 4][:, j % 4, :], 1.0,
                    knat[j // 4][:, j % 4, :],
                    op0=OP.mult, op1=OP.mult,
                    accum_out=ssq_k[:, j : j + 1],
                )
            vp = []
            for st in range(NS):
                t_ = main.tile([P, H, DH + 1], bf16, tag=f"vp{st}", name=f"vp{st}")
                nc.gpsimd.memset(t_[:, :, DH : DH + 1], 1.0)
                vp.append(t_)

            # ---------- qT[d, t] = Wqp @ query.T (exact fp32) ----------
            qT = []
            for dt_i in range(ND):
                t_ = main.tile([P, T], f32, tag=f"qt{dt_i}", name=f"q{dt_i}")
                ps = psW.tile(
                    [P, T], f32, tag="mmA" if dt_i % 2 == 0 else "mmB",
                    name=f"psq{dt_i}",
                )
                for c in range(3):
                    nc.tensor.matmul(
                        ps, lhsT=wqpT[c][:, ts(dt_i, P)], rhs=qryT[c][:],
                        start=(c == 0), stop=(c == 2),
                    )
                if dt_i == 2:
                    # rsq transposes slot in here (emitted below, PE order)
                    pass
                nc.scalar.copy(t_[:], ps)
                qT.append(t_)
                if dt_i == 2:
                    nc.scalar.activation(
                        rsq_k[:], ssq_k[:], AF.Sqrt, bias=eps12[:]
                    )
            nc.vector.reciprocal(rsq_k[:], rsq_k[:])

            # rsq_k columns -> one [1, S] row (PE transposes into psatt banks)
            ps_rowA = psatt.tile([1, T], f32, tag="att", name="ps_rowA")
            # (psatt tag must stay [P, T]-sized so 4 bufs fit in 4 banks)
            for j in range(4):
                nc.tensor.matmul(
                    ps_rowA[0:1, ts(j, P)], lhsT=rsq_k[:, j : j + 1], rhs=ident_f,
                    is_transpose=True, skip_group_check=True,
                )
            ps_rowB = psatt.tile([1, T], f32, tag="att", name="ps_rowB")
            for j in range(4):
                nc.tensor.matmul(
                    ps_rowB[0:1, ts(j, P)], lhsT=rsq_k[:, 4 + j : 5 + j],
                    rhs=ident_f, is_transpose=True, skip_group_check=True,
                )
            nc.vector.tensor_copy(rsq_row[0:1, 0:T], ps_rowA)
            nc.vector.tensor_copy(rsq_row[0:1, T : 2 * T], ps_rowB)
            # rsqB broadcast lives in PSUM only; the rescale reads it there
            ps_rsh = []
            for half in range(2):
                ps_h = psatt.tile([P, T], f32, tag="att", name=f"ps_rs{half}")
                nc.tensor.matmul(
                    ps_h, lhsT=ones_row,
                    rhs=rsq_row[0:1, ds(half * T, T)],
                    start=True, stop=True,
                )
                ps_rsh.append(ps_h)
            # keys l2-normalized in place (exact; feeds scores), per half so
            # the first score matmul isn't gated on the second half
            for half in range(2):
                for dc in range(ND):
                    nc.vector.tensor_tensor(
                        ktb[:, dc, ds(half * T, T)], ktb[:, dc, ds(half * T, T)],
                        ps_rsh[half], OP.mult,
                    )
            # rounded f32r copies for qhT (DVE, idle window)
            qTr_tags = ["qry0", "qry1", "qry2", "wqp0"]
            qTr = []
            for dt_i in range(ND):
                t_ = main.tile([P, T], f32r, tag=qTr_tags[dt_i], name=f"qr{dt_i}")
                nc.vector.tensor_copy(t_[:], qT[dt_i][:])
                qTr.append(t_)
            # rounded f32r copies of normalized keys for KpT (Pool, off-path)
            ktr_tags = ["kn0", "kn1", "wqp1", "wqp2"]
            ktr = []
            for dc in range(ND):
                t_ = main.tile([P, S], f32r, tag=ktr_tags[dc], name=f"ktr{dc}")
                nc.gpsimd.tensor_copy(t_[:], keysT[dc])
                ktr.append(t_)

            # ---------- scores[t, s] = q @ keysn.T (exact fp32) --------------
            sc = []
            for tt in range(NT):
                t_ = main.tile([P, S], f32, tag=f"sc{tt}", name=f"sc{tt}")
                for half in range(2):
                    ps = psW.tile(
                        [P, T], f32, tag="mmA" if half == 0 else "mmB",
                        name=f"pssc{tt}_{half}",
                    )
                    for dc in range(ND):
                        nc.tensor.matmul(
                            ps,
                            lhsT=qT[dc][:, ts(tt, P)],
                            rhs=keysT[dc][ds(0, P), ds(half * T, T)],
                            start=(dc == 0), stop=(dc == ND - 1),
                        )
                    nc.scalar.copy(t_[:, ds(half * T, T)], ps)
                sc.append(t_)

            # ---------- top-32 threshold per token row (DVE), mask on Pool ---
            mask01 = []
            for tt in range(NT):
                work = main.tile([P, S], f32, tag="tkwork", name=f"wk{tt}")
                cur = sc[tt]
                for r in range(4):
                    mx = const.tile([P, 8], f32, tag=f"mx{tt}_{r}")
                    nc.vector.max(out=mx[:], in_=cur[:])
                    if r < 3:
                        nc.vector.match_replace(
                            out=work[:], in_to_replace=mx[:], in_values=cur[:],
                            imm_value=-1e30,
                        )
                        cur = work
                m_ = main.tile([P, S], bf16, tag=f"mk{tt}", name=f"mk{tt}")
                eng = nc.vector if tt == NT - 1 else nc.gpsimd
                eng.tensor_scalar(
                    m_[:], sc[tt][:], mx[:, 7:8], None, op0=OP.is_ge
                )
                mask01.append(m_)

            # ---------- vals ssq (ACT, after the sc copies; DVE recip sits
            # after top-k so it never blocks the selection chain) ----------
            for j in range(NS):
                scr = sq2.tile([P, D], f32, tag="sq")
                nc.scalar.activation(
                    scr[:], vnb[:, j, :], AF.Square, accum_out=ssq_v[:, j : j + 1]
                )
            nc.scalar.activation(rsq_v[:], ssq_v[:], AF.Sqrt, bias=eps12[:])
            nc.vector.reciprocal(rsq_v[:], rsq_v[:])
            # preload the exp table set off the critical path
            nc.scalar.activation(dummy[:], dummy[:], AF.Exp)

            # ---------- qhT[e, t] = (Wq @ qT) / 8  (f32r) ----------
            qhT = []
            for e in range(ND):
                t_ = main.tile([P, T], f32r, tag=f"qh{e}", name=f"qh{e}")
                ps = psW.tile(
                    [P, T], f32, tag="mmA" if e % 2 == 0 else "mmB",
                    name=f"psqh{e}",
                )
                for dc in range(ND):
                    nc.tensor.matmul(
                        ps, lhsT=wqT[dc][:, ts(e, P)], rhs=qTr[dc][:],
                        start=(dc == 0), stop=(dc == ND - 1),
                    )
                nc.scalar.mul(t_[:], ps, 1.0 / np.sqrt(DH))
                qhT.append(t_)

            # ---------- KpT[e, s] = Wk @ keysn.T  (f32r) ----------
            kpT = []
            for e in range(ND):
                t_ = main.tile([P, S], f32r, tag=f"kp{e}", name=f"kp{e}")
                for half in range(2):
                    ps = psW.tile(
                        [P, T], f32, tag="mmA" if half == 0 else "mmB",
                        name=f"pskp{e}_{half}",
                    )
                    for dc in range(ND):
                        nc.tensor.matmul(
                            ps,
                            lhsT=wkT[dc][:, ts(e, P)],
                            rhs=ktr[dc][:, ds(half * T, T)],
                            start=(dc == 0), stop=(dc == ND - 1),
                        )
                    nc.scalar.copy(t_[:, ds(half * T, T)], ps)
                kpT.append(t_)

            # ---------- mask transpose to [s, t] (bf16, DVE copies) ----------
            mTh = [
                main.tile([P, 4, T], bf16, tag=f"sc{j}", name=f"mTh{j}")
                for j in range(2)
            ]

            def mT(c):
                return mTh[c // 4][:, c % 4, :]

            # ---------- brow[q] = bout + Wout @ ln_b (row layout) ----------
            ps_bc = psW.tile([1, QD], f32, tag="mmA")
            for e in range(ND):
                nc.tensor.matmul(
                    ps_bc, lhsT=b_sb[:, e : e + 1], rhs=woutT[e],
                    start=(e == 0), stop=(e == ND - 1),
                )
            nc.vector.tensor_tensor(
                brow[0:1, 0:QD], bout_row[0:1, 0:QD], ps_bc, OP.add
            )

            # ---------- Vp[s, 8 heads x (64 + ones)] = valsn @ Wv.T (bf16) ---
            # raw vals in the matmul; the per-slot 1/||v|| lands on the copy.
            def emit_vp(st):
                ps = psW.tile(
                    [P, D], f32, tag="mmA" if st % 2 == 0 else "mmB",
                    name=f"psvp{st}",
                )
                for dc in range(ND):
                    nc.tensor.matmul(
                        ps,
                        lhsT=valsT[dc][:, ts(st, P)],
                        rhs=wvT[dc][:],
                        start=(dc == 0), stop=(dc == ND - 1),
                    )
                if st < 5:
                    nc.scalar.activation(
                        vp[st][:, :, 0:DH],
                        ps.rearrange("p (h e) -> p h e", h=H),
                        AF.Copy, scale=rsq_v[:, st : st + 1],
                    )
                else:
                    nc.vector.tensor_scalar(
                        vp[st][:, :, 0:DH],
                        ps.rearrange("p (h e) -> p h e", h=H),
                        rsq_v[:, st : st + 1], None, op0=OP.mult,
                    )

            emit_vp(0)
            emit_vp(1)

            # ---------- attention: masked softmax over all S slots -----------
            # unmerged single-chunk units with 4 PSUM buffers: deep runway so
            # the ACT exp stream never waits on the PE ctx/w chain.
            ctx_tok = main.tile([P, NT, T], bf16, tag="wkb", name="cxb")
            ctxT_big = main.tile([P, ND, T], f32r, tag="ctxT", name="ctxT")
            uA = main.tile([P, 4, T], bf16, tag="kn0", name="uA")
            uB = main.tile([P, 2, T], bf16, tag="kn1", name="uB")
            # slots 0-4: DVE w ring; slots 5,6: deferred Pool units (c=1, c=5)
            wbig = main.tile([P, 8, T], bf16, tag="vnb", name="wbig")

            def u_slot(i):
                m = i % 6
                if m < 4:
                    return uA[:, m, :]
                return uB[:, m - 4, :]

            units = [(h, c) for h in range(H) for c in range(NS)]
            u_of = {}
            ps_ctx_of = {}
            deferred = {}
            widx = [0]
            mT_done = [0]

            def att_logits(i):
                h, c = units[i]
                if c == 0:
                    ps_ctx_of[h] = psctx.tile(
                        [P, NT, DH + 1], f32, tag="ctx", name=f"ctx{h}"
                    )
                    nc.vector.memset(ps_ctx_of[h][:], 0.0)
                    deferred[h] = []
                e, ro = h // 2, (h % 2) * DH
                ps = psatt.tile([P, T], f32, tag="att", name=f"att{h}_{c}")
                nc.tensor.matmul(
                    ps,
                    lhsT=kpT[e][ro : ro + DH, ts(c, P)],
                    rhs=qhT[e][ro : ro + DH, :],
                    start=True, stop=True,
                )
                u = u_slot(i)
                nc.scalar.activation(u, ps, AF.Exp)
                u_of[i] = u

            def ctx_mms(h, c, w, stop):
                ps_ctx = ps_ctx_of[h]
                for tt in range(NT):
                    nc.tensor.matmul(
                        ps_ctx[:, tt, :],
                        lhsT=w[:, ts(tt, P)],
                        rhs=vp[c][:, h, :],
                        start=False, stop=stop,
                        skip_group_check=True,
                    )

            def att_wctx(i):
                h, c = units[i]
                u = u_of.pop(i)
                # h0: transpose + copy each mask chunk just before its first
                # use, pipelined through the psW ring
                if h == 0 and mT_done[0] <= c:
                    ps_t = psW.tile(
                        [P, T], bf16, tag="mmA" if c % 2 == 0 else "mmB",
                        name=f"psmT{c}",
                    )
                    for tt in range(NT):
                        nc.tensor.matmul(
                            ps_t[:, ts(tt, P)], lhsT=mask01[tt][:, ts(c, P)],
                            rhs=ident, is_transpose=True, skip_group_check=True,
                        )
                    nc.vector.tensor_copy(mT(c), ps_t)
                    mT_done[0] = c + 1
                if c in (1, 5) or (c == 3 and h % 2 == 1):
                    # Pool unit: slow mult, ctx matmuls deferred to the end of
                    # the head so the in-order PE never waits on it
                    w = wbig[:, 5 + (1 if c == 5 else (2 if c == 3 else 0)), :]
                    nc.gpsimd.tensor_tensor(w, u, mT(c), OP.mult)
                    deferred[h].append((c, w))
                else:
                    w = wbig[:, widx[0] % 5, :]
                    widx[0] += 1
                    nc.vector.tensor_tensor(w, u, mT(c), OP.mult)
                    ctx_mms(h, c, w, stop=False)
                if c == NS - 1:
                    for idx, (dc_, dw) in enumerate(deferred[h]):
                        ctx_mms(h, dc_, dw, stop=(idx == len(deferred[h]) - 1))
                    drain_head(h)
                    if h % 2 == 1:
                        wo_pair(h // 2)

            def drain_head(h):
                ps_ctx = ps_ctx_of.pop(h)
                rcp = const.tile([P, NT], f32, tag=f"rcp{h % 2}", name=f"rcp{h}")
                nc.vector.reciprocal(rcp[:], ps_ctx[:, :, DH : DH + 1])
                for tt in range(NT):
                    nc.vector.tensor_scalar(
                        ctx_tok[:, tt, ds(h * DH, DH)],
                        ps_ctx[:, tt, 0:DH],
                        rcp[:, tt : tt + 1], None, op0=OP.mult,
                    )

            def wo_pair(e):
                # transpose ctx cols for heads (2e, 2e+1) into ctxT_big[:, e, :]
                ps_t = psW.tile(
                    [P, T], bf16, tag="mmA" if e % 2 == 0 else "mmB", name=f"pt{e}"
                )
                for tt in range(NT):
                    nc.tensor.matmul(
                        ps_t[:, ts(tt, P)], lhsT=ctx_tok[:, tt, ts(e, P)],
                        rhs=ident, is_transpose=True, skip_group_check=True,
                    )
                nc.vector.tensor_copy(ctxT_big[:, e, :], ps_t)

            # software pipeline, depth 4 to match the psum ring; first units
            # zippered with the remaining Vp matmuls
            att_logits(0)
            emit_vp(2)
            att_logits(1)
            emit_vp(3)
            att_logits(2)
            emit_vp(4)
            att_logits(3)
            att_wctx(0)
            emit_vp(5)
            att_logits(4)
            att_wctx(1)
            emit_vp(6)
            att_logits(5)
            att_wctx(2)
            emit_vp(7)
            for i in range(6, len(units)):
                att_logits(i)
                att_wctx(i - 3)
            att_wctx(len(units) - 3)
            att_wctx(len(units) - 2)
            att_wctx(len(units) - 1)

            # fold ln_g into Wout rows (Pool; WoutT bconst reads are done)
            for e in range(ND):
                nc.gpsimd.tensor_scalar(
                    woutT[e], woutT[e], g_sb[:, e : e + 1], None, op0=OP.mult
                )

            # preload the sqrt table while the last heads drain
            nc.scalar.activation(dummy[:], dummy[:], AF.Sqrt)

            # ---------- o[t, e] = ctx.T @ Wo.T (token-major), LN row-wise ----
            # per-tt pipeline: Wo -> (ACT copy + mu accum) || (DVE ssq from
            # PSUM) -> tiny stats -> xhat, so tile 0 finishes while tile 3
            # is still in the matmul.
            o_big = main.tile([P, NT, D], f32, tag="ktb", name="o_big")
            xh_big = main.tile([P, NT, D], bf16, tag="vtb", name="xh_big")
            for tt in range(NT):
                ps_o = psW.tile(
                    [P, D], f32, tag="mmA" if tt % 2 == 0 else "mmB",
                    name=f"pso{tt}",
                )
                for dc in range(ND):
                    nc.tensor.matmul(
                        ps_o,
                        lhsT=ctxT_big[:, dc, ts(tt, P)],
                        rhs=woT[dc][:],
                        start=(dc == 0), stop=(dc == ND - 1),
                    )
                nc.scalar.activation(
                    o_big[:, tt, :], ps_o, AF.Copy, accum_out=mu_c[:, tt : tt + 1]
                )
                scr = sq2.tile([P, D], f32, tag="sq")
                nc.scalar.activation(
                    scr[:], o_big[:, tt, :], AF.Square,
                    accum_out=ssq_c[:, tt : tt + 1],
                )
                ttc = ds(tt, 1)
                nc.vector.tensor_scalar(
                    mu2_c[:, ttc], mu_c[:, ttc], 1.0 / D, None, op0=OP.mult
                )
                nc.vector.tensor_tensor(
                    var_c[:, ttc], mu2_c[:, ttc], mu2_c[:, ttc], OP.mult
                )
                nc.vector.scalar_tensor_tensor(
                    var_c[:, ttc], ssq_c[:, ttc], 1.0 / D, var_c[:, ttc],
                    op0=OP.mult, op1=OP.subtract,
                )
                nc.scalar.activation(
                    rstd_c[:, ttc], var_c[:, ttc], AF.Sqrt, bias=eps_ln[:]
                )
                nc.vector.reciprocal(rstd_c[:, ttc], rstd_c[:, ttc])
                nc.vector.scalar_tensor_tensor(
                    xh_big[:, tt, :], o_big[:, tt, :], mu2_c[:, tt : tt + 1],
                    rstd_c[:, tt : tt + 1].to_broadcast([P, D]),
                    op0=OP.subtract, op1=OP.mult,
                )
            xhatT_big = main.tile([P, ND, T], f32r, tag="xhT", name="xhT")
            for e in range(ND):
                ps_t = psW.tile(
                    [P, T], bf16, tag="mmA" if e % 2 == 0 else "mmB",
                    name=f"psxT{e}",
                )
                for tt in range(NT):
                    nc.tensor.matmul(
                        ps_t[:, ts(tt, P)], lhsT=xh_big[:, tt, ts(e, P)],
                        rhs=ident, is_transpose=True, skip_group_check=True,
                    )
                if e % 2 == 0:
                    nc.vector.tensor_copy(xhatT_big[:, e, :], ps_t)
                else:
                    nc.scalar.copy(xhatT_big[:, e, :], ps_t)

            # ---------- outT[q, t] = (g*Wout).T @ xhatT + brow ----------
            for qt, (off, sz) in enumerate(QD_TILES):
                ps = psW.tile(
                    [P, T], f32, tag="mmA" if qt % 2 == 0 else "mmB",
                    name=f"psout{qt}",
                )
                for e in range(ND):
                    nc.tensor.matmul(
                        ps[:sz, :], lhsT=woutT[e][:, ds(off, sz)],
                        rhs=xhatT_big[:, e, :],
                        start=(e == 0), stop=False,
                    )
                nc.tensor.matmul(
                    ps[:sz, :], lhsT=brow[0:1, ds(off, sz)], rhs=ones_rT[:],
                    start=False, stop=True,
                )
                ot_sb = sq2.tile([P, T], f32, tag="sq", name=f"ot{qt}")
                nc.scalar.copy(ot_sb[:sz, :], ps[:sz, :])
                nc.sync.dma_start(out_dram.ap()[ds(off, sz), :], ot_sb[:sz, :])

    nc.compile()
    return nc


def _prep_in_maps(inputs):
    import ml_dtypes

    def c(a):
        return np.ascontiguousarray(a, dtype=np.float32)

    q = np.asarray(inputs["query_states"], dtype=np.float32).reshape(B * N, QD)
    vals = np.asarray(inputs["mem_values"], dtype=np.float32)
    shared = {
        "WqpT": c(np.asarray(inputs["Wqp"]).T),
        "WqT": c(np.asarray(inputs["Wq"]).T),
        "WkT": c(np.asarray(inputs["Wk"]).T),
        "WvT": c(np.asarray(inputs["Wv"]).T),
        "WoT": c(np.asarray(inputs["Wo"]).T),
        "WoutT": c(np.asarray(inputs["Wout"]).T),
        "memk_nat": c(np.asarray(inputs["mem_keys"])),
        "memkT": c(np.asarray(inputs["mem_keys"]).T),
        "memv_nat": np.ascontiguousarray(vals.astype(ml_dtypes.bfloat16)),
        "memvT": c(vals.T),
        "ln_g": c(np.asarray(inputs["ln_g"])),
        "ln_b": c(np.asarray(inputs["ln_b"])),
        "bout": c(np.pad(np.asarray(inputs["bout"]), (0, 384 - QD))),
    }
    in_maps = []
    for core in range(NCORES):
        m = dict(shared)
        m["queryT"] = c(q[core * T : (core + 1) * T, :].T)
        in_maps.append(m)
    return in_maps


def kernel(**inputs) -> np.ndarray:
    if "nc" not in _CACHE:
        _CACHE["nc"] = _build_nc()
    nc = _CACHE["nc"]
    in_maps = _prep_in_maps(inputs)
    res = run_bass_kernel_spmd(nc, in_maps, core_ids=list(range(NCORES)))
    out = np.empty((B * N, QD), dtype=np.float32)
    for core in range(NCORES):
        out[core * T : (core + 1) * T, :] = res.results[core]["outT"].T
    return out.reshape(B, N, QD)
